# revision 6
# baseline (speedup 1.0000x reference)
"""Trainium2 Bass kernel for nn_AdaBoostClassifier (8-core data-parallel).

Reference computation:
    logits = x @ W.T + b                      # [N, E]
    preds  = round(sigmoid(logits))           # {0,1} == 1[logit > 0]
    acc    = sum_e trunc(alpha_e) * preds_e   # integer-valued
    out    = sign(acc)

Math: with t = trunc(alphas): acc = dot(t, preds), preds_e = 1[logit_e > 0]
(is_gt matches round-half-even at the boundary). Columns with t_e == 0
contribute nothing, so only those estimators are computed (selected on host
at runtime — valid for any input values).

Device pipeline (v3), per 512-sample block, all in ONE PSUM accumulation
group scaled by 2^12:
  4x fp16 matmul   xh @ (Wh * 2^12)              = 2^12 * xh@Wh
  4x fp8 DoubleRow [xh8; xl*2^12] @ [Wl*2^12; Wh8] = 2^12 * (x@W - xh@Wh)
  => PSUM ~= 2^12 * x@W.  pred = 1[PSUM > -2^12 b] on DVE (single op).
  acc = t . preds via PE matvec (t broadcast to 32 cols on host; 4
  consecutive blocks fill two [64] acc banks at rows 0|32), two half-Signs
  per group on ACT, one 4-row strided DMA out.

x ships as two planes repacked on host to [128, 32 blocks, KC, 512] so
every (partition, block) line is contiguous (4KB fp16 / 2KB fp8) and any
run of blocks is one large-descriptor DMA. The stream is chunked
small->large->small (ramp schedule) to fill and drain the pipeline fast.
Matmul-critical weights ride at the head of the same queue as xh so they
cannot be starved by the x backlog.
"""

import math
import os

import numpy as np
import ml_dtypes

import concourse.bass as bass  # noqa: F401  (registers bass types)
import concourse.tile as tile
from concourse import bacc, mybir
from concourse.bass_utils import run_bass_kernel_spmd

BF16 = ml_dtypes.bfloat16
F8E4 = ml_dtypes.float8_e4m3

XBUFS = int(os.environ.get("KERNEL_XBUFS", "5"))
SPOOL = int(os.environ.get("KERNEL_SPOOL", "8"))
PSLOG = int(os.environ.get("KERNEL_PSLOG", "6"))
MMORD = os.environ.get("KERNEL_MMORD", "block")  # block | phase
PEND = int(os.environ.get("KERNEL_PEND", "4"))
WARM = int(os.environ.get("KERNEL_WARM", "10"))
WARMC = int(os.environ.get("KERNEL_WARMC", "128"))
XL_SCALE = 2.0 ** 12
XL_INV_SCALE = 1.0 / XL_SCALE

N_CORES = 8
N_FULL = 131072
F_DIM = 512
NS = N_FULL // N_CORES          # samples per core
BLK = 512                       # samples per psum block (one PSUM bank)
NBLK = NS // BLK                # 32 blocks per core
KC = F_DIM // 128               # contraction chunks
GRP = 4                         # blocks per acc/Sign group
# DMA chunk schedule in blocks: ramp up, stream, ramp down
_SCHED = os.environ.get("KERNEL_SCHED", "1,1,2,4,4,4,4,4,4,2,1,1")
SCHED = [int(v) for v in _SCHED.split(",")]
assert sum(SCHED) == NBLK
CHUNK_MAX = max(SCHED)

_program_cache: dict[tuple, object] = {}


def _build(n_etiles: int):
    """Build the 8-core SPMD program for n_etiles 128-wide estimator tiles."""
    nc = bacc.Bacc("TRN2", target_bir_lowering=False, debug=False)

    d_xh = nc.dram_tensor(
        "xh", [128, NBLK, KC, BLK], mybir.dt.float16, kind="ExternalInput"
    )
    d_xl = nc.dram_tensor(
        "xl", [128, NBLK, KC, BLK], mybir.dt.float8e4, kind="ExternalInput"
    )
    # weights pre-repacked on host to the SBUF layout: per-partition
    # contiguous so each is one 128-descriptor DMA
    d_wh2 = nc.dram_tensor(
        "wh2", [128, n_etiles, KC, 128], mybir.dt.float16, kind="ExternalInput"
    )
    d_w8 = nc.dram_tensor(
        "w8", [128, n_etiles, KC, 2, 128], mybir.dt.float8e4, kind="ExternalInput"
    )
    d_bv2 = nc.dram_tensor(
        "bv2", [128, n_etiles], mybir.dt.float32, kind="ExternalInput"
    )
    d_tvb = nc.dram_tensor(
        "tvb", [128, n_etiles, 32], mybir.dt.bfloat16, kind="ExternalInput"
    )
    d_out = nc.dram_tensor("out", [NS], mybir.dt.float32, kind="ExternalOutput")

    xh_v = d_xh.ap()
    xl_v = d_xl.ap()
    out_v = d_out.ap().rearrange("(g s n) -> g s n", s=GRP, n=BLK)

    with tile.TileContext(nc) as tc:
        with (
            tc.tile_pool(name="singles", bufs=1) as singles,
            tc.tile_pool(name="xbuf", bufs=XBUFS) as xbuf,
            tc.tile_pool(name="sbuf", bufs=SPOOL) as spool,
            tc.tile_pool(name="obuf", bufs=2) as obuf,
            tc.tile_pool(name="pslog", bufs=PSLOG, space="PSUM") as pslog,
            tc.tile_pool(name="psacc", bufs=2, space="PSUM") as psacc,
        ):
            # matmul-critical weights FIRST on the same (sync/SP) queue as
            # xh: in-queue FIFO guarantees they beat the x backlog
            wh2_t = singles.tile([128, n_etiles, KC, 128], mybir.dt.float16,
                                 tag="wh2")
            nc.sync.dma_start(out=wh2_t, in_=d_wh2.ap())
            w8_t = singles.tile([128, n_etiles, KC, 2, 128], mybir.dt.float8e4,
                                tag="w8")
            nc.sync.dma_start(out=w8_t, in_=d_w8.ap())
            # small late-use constants on the scalar ring
            bv2_t = singles.tile([128, n_etiles], mybir.dt.float32, tag="bv2")
            nc.scalar.dma_start(out=bv2_t, in_=d_bv2.ap())
            tvb_t = singles.tile([128, n_etiles, 32], mybir.dt.bfloat16,
                                 tag="tvb")
            nc.scalar.dma_start(out=tvb_t, in_=d_tvb.ap())

            # PE warmup: short dummy matmuls while the first x chunk streams
            # in — ramps the HAM clock gate toward 2.4 GHz without delaying
            # the first real matmul much.
            if WARM:
                wsrc = singles.tile([128, WARMC], mybir.dt.bfloat16, tag="wsrc")
                nc.vector.memset(wsrc, 0.0)
                warmp = psacc.tile([64, WARMC], mybir.dt.float32, tag="acc",
                                   name="warmp")
                for _ in range(WARM):
                    nc.tensor.matmul(
                        warmp, wsrc[:, 0:64], wsrc, start=True, stop=True
                    )

            # --- main loop over DMA chunks / compute blocks ---
            pending = []  # (s_tiles per j, global block index)
            acc_tiles = {}
            o_tiles = {}

            def flush_stage2():
                # matvec base partitions are limited to {0, 32, 64}: the 4
                # accs of a group go into two [64, BLK] banks at rows 0|32.
                s_ts, bi = pending.pop(0)
                g, pos = bi // GRP, bi % GRP
                bank, row = pos // 2, 32 * (pos % 2)
                if row == 0:
                    acc_tiles[(g, bank)] = psacc.tile(
                        [64, BLK], mybir.dt.float32, tag="acc",
                        name=f"acc{g}_{bank}"
                    )
                acc = acc_tiles[(g, bank)]
                for j in range(n_etiles):
                    nc.tensor.matmul(
                        acc[row:row + 32, :], tvb_t[:, j], s_ts[j],
                        start=(j == 0), stop=(j == n_etiles - 1),
                    )
                if row == 32:
                    if bank == 0:
                        o_tiles[g] = obuf.tile(
                            [128, BLK], mybir.dt.float32, tag="osb",
                            name=f"osb{g}")
                    o_g = o_tiles[g]
                    nc.scalar.activation(
                        out=o_g[64 * bank:64 * bank + 64, :], in_=acc,
                        func=mybir.ActivationFunctionType.Sign,
                    )
                    if bank == 1:
                        nc.scalar.dma_start(
                            out=out_v[g], in_=o_g[31:128:32, :]
                        )

            b0 = 0
            for ci, nb in enumerate(SCHED):
                xh_sb = xbuf.tile([128, CHUNK_MAX, KC, BLK], mybir.dt.float16,
                                  tag="xh", name=f"xh{ci}")
                x8_sb = xbuf.tile([128, 2, CHUNK_MAX, KC, BLK],
                                  mybir.dt.float8e4, tag="x8", name=f"x8{ci}")
                nc.sync.dma_start(
                    out=xh_sb[:, 0:nb], in_=xh_v[:, b0:b0 + nb])
                nc.gpsimd.dma_start(
                    out=x8_sb[:, 1, 0:nb], in_=xl_v[:, b0:b0 + nb])
                # fill fp8 pair plane 0 with fp8(xh); split across DVE and ACT
                half = KC // 2
                nc.vector.tensor_copy(
                    out=x8_sb[:, 0, 0:nb, 0:half, :],
                    in_=xh_sb[:, 0:nb, 0:half, :])
                nc.scalar.activation(
                    out=x8_sb[:, 0, 0:nb, half:KC, :],
                    in_=xh_sb[:, 0:nb, half:KC, :],
                    func=mybir.ActivationFunctionType.Copy)

                # "phase": all fp16 matmuls of the chunk first (they only
                # need xh) so the PE covers the fp8-cast latency; "block":
                # interleave per block for steady PSUM-bank churn.
                lg = {
                    s: [
                        pslog.tile([128, BLK], mybir.dt.float32, tag="lg",
                                   name=f"lg{b0 + s}_{j}")
                        for j in range(n_etiles)
                    ]
                    for s in range(nb)
                }
                if MMORD == "phase":
                    for k in range(KC):
                        for s in range(nb):
                            for j in range(n_etiles):
                                nc.tensor.matmul(
                                    lg[s][j], wh2_t[:, j, k], xh_sb[:, s, k, :],
                                    start=(k == 0), stop=False,
                                )
                    for k in range(KC):
                        for s in range(nb):
                            for j in range(n_etiles):
                                nc.tensor.matmul(
                                    lg[s][j], w8_t[:, j, k], x8_sb[:, :, s, k, :],
                                    start=False, stop=(k == KC - 1),
                                    perf_mode=mybir.MatmulPerfMode.DoubleRow,
                                )
                else:
                    for s in range(nb):
                        for j in range(n_etiles):
                            for k in range(KC):
                                nc.tensor.matmul(
                                    lg[s][j], wh2_t[:, j, k], xh_sb[:, s, k, :],
                                    start=(k == 0), stop=False,
                                )
                            for k in range(KC):
                                nc.tensor.matmul(
                                    lg[s][j], w8_t[:, j, k], x8_sb[:, :, s, k, :],
                                    start=False, stop=(k == KC - 1),
                                    perf_mode=mybir.MatmulPerfMode.DoubleRow,
                                )
                for s in range(nb):
                    bi = b0 + s
                    s_ts = []
                    for j in range(n_etiles):
                        s_t = spool.tile([128, BLK], mybir.dt.bfloat16,
                                         tag="sg", name=f"sg{bi}_{j}")
                        # pred = 1[2^12 x@W > -2^12 b]
                        nc.vector.tensor_scalar(
                            s_t, lg[s][j], bv2_t[:, j:j + 1], None,
                            mybir.AluOpType.is_gt,
                        )
                        s_ts.append(s_t)
                    pending.append((s_ts, bi))
                    while len(pending) > PEND:
                        flush_stage2()
                b0 += nb
            while pending:
                flush_stage2()

    nc.compile()
    return nc


def _prep_inputs(x, W, b, alphas):
    """Host-side prep: estimator selection, transposes, hi/lo splits."""
    t_full = np.trunc(alphas.astype(np.float32)).astype(np.float32)
    nz = np.flatnonzero(t_full)
    n_etiles = max(1, math.ceil(len(nz) / 128))
    e_pad = n_etiles * 128

    W_sel = np.zeros((e_pad, F_DIM), np.float32)
    b_sel = np.zeros((e_pad,), np.float32)
    t_sel = np.zeros((e_pad,), np.float32)
    if len(nz):
        W_sel[: len(nz)] = W[nz]
        b_sel[: len(nz)] = b[nz]
        t_sel[: len(nz)] = t_full[nz]

    # [n_etiles, F, 128] stationary layout (partition = feature), then
    # repacked to the SBUF tile layout [128p, j, k, e]
    w_fe = W_sel.T.reshape(F_DIM, n_etiles, 128).transpose(1, 0, 2)
    wh = w_fe.astype(np.float16)
    # exact power-of-two scale: fp16(W) * 2^12 has no extra rounding
    wh2 = (wh.astype(np.float32) * XL_SCALE).astype(np.float16)
    wl32 = w_fe - wh.astype(np.float32)
    w8 = np.empty((n_etiles, F_DIM, 2, 128), F8E4)
    w8[..., 0, :] = (wl32 * XL_SCALE).astype(F8E4)
    w8[..., 1, :] = wh.astype(np.float32).astype(F8E4)
    # [j, (k p), ...] -> [p, j, k, ...]
    wh2_p = np.ascontiguousarray(
        wh2.reshape(n_etiles, KC, 128, 128).transpose(2, 0, 1, 3))
    w8_p = np.ascontiguousarray(
        w8.reshape(n_etiles, KC, 128, 2, 128).transpose(2, 0, 1, 3, 4))

    bv2 = np.ascontiguousarray(
        (-(b_sel * XL_SCALE)).reshape(n_etiles, 128).T
    ).astype(np.float32)
    tvb = np.ascontiguousarray(
        np.broadcast_to(
            t_sel.reshape(n_etiles, 128, 1), (n_etiles, 128, 32)
        ).transpose(1, 0, 2)
    ).astype(BF16)

    xT = np.ascontiguousarray(x.T.astype(np.float32))  # [F, N]
    xh = xT.astype(np.float16)
    xl = ((xT - xh.astype(np.float32)) * XL_SCALE).astype(F8E4)

    in_maps = []
    for c in range(N_CORES):
        sl = slice(c * NS, (c + 1) * NS)
        # repack: [F=(k p), n] -> [p, block, k, n] so every (p, block) line
        # is contiguous and any run of blocks is one large-descriptor DMA
        xh_c = xh[:, sl].reshape(KC, 128, NBLK, BLK).transpose(1, 2, 0, 3)
        xl_c = xl[:, sl].reshape(KC, 128, NBLK, BLK).transpose(1, 2, 0, 3)
        in_maps.append({
            "xh": np.ascontiguousarray(xh_c),
            "xl": np.ascontiguousarray(xl_c),
            "wh2": wh2_p, "w8": w8_p, "bv2": bv2, "tvb": tvb,
        })
    return n_etiles, in_maps


def kernel(x, W, b, alphas, _trace=False, _trace_kwargs=None):
    n_etiles, in_maps = _prep_inputs(
        np.asarray(x), np.asarray(W), np.asarray(b), np.asarray(alphas)
    )
    cache_key = (n_etiles, _SCHED, XBUFS, SPOOL, PSLOG, PEND, WARM, WARMC, MMORD)
    nc = _program_cache.get(cache_key)
    if nc is None:
        nc = _build(n_etiles)
        _program_cache[cache_key] = nc

    kwargs = {}
    if _trace:
        kwargs["trace"] = True
        kwargs.update(_trace_kwargs or {})
    res = run_bass_kernel_spmd(nc, in_maps, core_ids=list(range(N_CORES)), **kwargs)
    out = np.concatenate([res.results[c]["out"] for c in range(N_CORES)])
    if _trace:
        kernel.last_results = res
    return out.astype(np.float32)


# revision 7
# speedup vs baseline: 1.1135x; 1.1135x over previous
"""Trainium2 Bass kernel for nn_AdaBoostClassifier (8-core data-parallel).

Reference computation:
    logits = x @ W.T + b                      # [N, E]
    preds  = round(sigmoid(logits))           # {0,1} == 1[logit > 0]
    acc    = sum_e trunc(alpha_e) * preds_e   # integer-valued
    out    = sign(acc)

Math: with t = trunc(alphas): acc = dot(t, preds), preds_e = 1[logit_e > 0]
(is_gt matches round-half-even at the boundary). Columns with t_e == 0
contribute nothing, so only those estimators are computed (selected on host
at runtime — valid for any input values).

Device pipeline (v3), per 512-sample block, all in ONE PSUM accumulation
group scaled by 2^12:
  4x fp16 matmul   xh @ (Wh * 2^12)              = 2^12 * xh@Wh
  4x fp8 DoubleRow [xh8; xl*2^12] @ [Wl*2^12; Wh8] = 2^12 * (x@W - xh@Wh)
  => PSUM ~= 2^12 * x@W.  pred = 1[PSUM > -2^12 b] on DVE (single op).
  acc = t . preds via PE matvec (t broadcast to 32 cols on host; 4
  consecutive blocks fill two [64] acc banks at rows 0|32), two half-Signs
  per group on ACT, one 4-row strided DMA out.

x ships as two planes repacked on host to [128, 32 blocks, KC, 512] so
every (partition, block) line is contiguous (4KB fp16 / 2KB fp8) and any
run of blocks is one large-descriptor DMA. The stream is chunked
small->large->small (ramp schedule) to fill and drain the pipeline fast.
Matmul-critical weights ride at the head of the same queue as xh so they
cannot be starved by the x backlog.
"""

import math
import os

import numpy as np
import ml_dtypes

import concourse.bass as bass  # noqa: F401  (registers bass types)
import concourse.tile as tile
from concourse import bacc, mybir
from concourse.bass_utils import run_bass_kernel_spmd

BF16 = ml_dtypes.bfloat16
F8E4 = ml_dtypes.float8_e4m3

XBUFS = int(os.environ.get("KERNEL_XBUFS", "4"))
SPOOL = int(os.environ.get("KERNEL_SPOOL", "6"))
PSLOG = int(os.environ.get("KERNEL_PSLOG", "5"))
MMORD = os.environ.get("KERNEL_MMORD", "block")  # block | phase
PEND = int(os.environ.get("KERNEL_PEND", "2"))
WARM = int(os.environ.get("KERNEL_WARM", "10"))
WARMC = int(os.environ.get("KERNEL_WARMC", "128"))
XL_SCALE = 2.0 ** 12
XL_INV_SCALE = 1.0 / XL_SCALE

N_CORES = 8
N_FULL = 131072
F_DIM = 512
NS = N_FULL // N_CORES          # samples per core
BLK = 512                       # samples per psum block (one PSUM bank)
NBLK = NS // BLK                # 32 blocks per core
KC = F_DIM // 128               # contraction chunks
GRP = 4                         # blocks per acc/Sign group
# DMA chunk schedule in blocks: ramp up, stream, ramp down
_SCHED = os.environ.get("KERNEL_SCHED", "1,1,2,4,4,4,4,4,4,2,1,1")
SCHED = [int(v) for v in _SCHED.split(",")]
assert sum(SCHED) == NBLK
CHUNK_MAX = max(SCHED)

_program_cache: dict[tuple, object] = {}


def _build(n_etiles: int):
    """Build the 8-core SPMD program for n_etiles 128-wide estimator tiles."""
    nc = bacc.Bacc("TRN2", target_bir_lowering=False, debug=False)

    d_xh = nc.dram_tensor(
        "xh", [128, NBLK, KC, BLK], mybir.dt.float16, kind="ExternalInput"
    )
    d_xl = nc.dram_tensor(
        "xl", [128, NBLK, KC, BLK], mybir.dt.float8e4, kind="ExternalInput"
    )
    # weights pre-repacked on host to the SBUF layout: per-partition
    # contiguous so each is one 128-descriptor DMA
    d_wh2 = nc.dram_tensor(
        "wh2", [128, n_etiles, KC, 128], mybir.dt.float16, kind="ExternalInput"
    )
    d_w8 = nc.dram_tensor(
        "w8", [128, n_etiles, KC, 2, 128], mybir.dt.float8e4, kind="ExternalInput"
    )
    d_bv2 = nc.dram_tensor(
        "bv2", [128, n_etiles], mybir.dt.float32, kind="ExternalInput"
    )
    d_tvb = nc.dram_tensor(
        "tvb", [128, n_etiles, 32], mybir.dt.bfloat16, kind="ExternalInput"
    )
    d_out = nc.dram_tensor("out", [NS], mybir.dt.float32, kind="ExternalOutput")

    xh_v = d_xh.ap()
    xl_v = d_xl.ap()
    out_v = d_out.ap().rearrange("(g s n) -> g s n", s=GRP, n=BLK)

    with tile.TileContext(nc) as tc:
        with (
            tc.tile_pool(name="singles", bufs=1) as singles,
            tc.tile_pool(name="xbuf", bufs=XBUFS) as xbuf,
            tc.tile_pool(name="sbuf", bufs=SPOOL) as spool,
            tc.tile_pool(name="obuf", bufs=2) as obuf,
            tc.tile_pool(name="pslog", bufs=PSLOG, space="PSUM") as pslog,
            tc.tile_pool(name="psacc", bufs=2, space="PSUM") as psacc,
        ):
            # matmul-critical weights FIRST on the same (sync/SP) queue as
            # xh: in-queue FIFO guarantees they beat the x backlog
            wh2_t = singles.tile([128, n_etiles, KC, 128], mybir.dt.float16,
                                 tag="wh2")
            nc.sync.dma_start(out=wh2_t, in_=d_wh2.ap())
            w8_t = singles.tile([128, n_etiles, KC, 2, 128], mybir.dt.float8e4,
                                tag="w8")
            nc.sync.dma_start(out=w8_t, in_=d_w8.ap())
            # small late-use constants on the scalar ring
            bv2_t = singles.tile([128, n_etiles], mybir.dt.float32, tag="bv2")
            nc.scalar.dma_start(out=bv2_t, in_=d_bv2.ap())
            tvb_t = singles.tile([128, n_etiles, 32], mybir.dt.bfloat16,
                                 tag="tvb")
            nc.scalar.dma_start(out=tvb_t, in_=d_tvb.ap())

            # PE warmup: short dummy matmuls while the first x chunk streams
            # in — ramps the HAM clock gate toward 2.4 GHz without delaying
            # the first real matmul much.
            if WARM:
                wsrc = singles.tile([128, WARMC], mybir.dt.bfloat16, tag="wsrc")
                nc.vector.memset(wsrc, 0.0)
                warmp = psacc.tile([64, WARMC], mybir.dt.float32, tag="acc",
                                   name="warmp")
                for _ in range(WARM):
                    nc.tensor.matmul(
                        warmp, wsrc[:, 0:64], wsrc, start=True, stop=True
                    )

            # --- main loop over DMA chunks / compute blocks ---
            pending = []  # (s_tiles per j, global block index)
            acc_tiles = {}
            o_tiles = {}

            def flush_stage2():
                # matvec base partitions are limited to {0, 32, 64}: the 4
                # accs of a group go into two [64, BLK] banks at rows 0|32.
                s_ts, bi = pending.pop(0)
                g, pos = bi // GRP, bi % GRP
                bank, row = pos // 2, 32 * (pos % 2)
                if row == 0:
                    acc_tiles[(g, bank)] = psacc.tile(
                        [64, BLK], mybir.dt.float32, tag="acc",
                        name=f"acc{g}_{bank}"
                    )
                acc = acc_tiles[(g, bank)]
                for j in range(n_etiles):
                    nc.tensor.matmul(
                        acc[row:row + 32, :], tvb_t[:, j], s_ts[j],
                        start=(j == 0), stop=(j == n_etiles - 1),
                    )
                if row == 32:
                    if bank == 0:
                        o_tiles[g] = obuf.tile(
                            [128, BLK], mybir.dt.float32, tag="osb",
                            name=f"osb{g}")
                    o_g = o_tiles[g]
                    nc.scalar.activation(
                        out=o_g[64 * bank:64 * bank + 64, :], in_=acc,
                        func=mybir.ActivationFunctionType.Sign,
                    )
                    if bank == 1:
                        nc.scalar.dma_start(
                            out=out_v[g], in_=o_g[31:128:32, :]
                        )

            b0 = 0
            for ci, nb in enumerate(SCHED):
                xh_sb = xbuf.tile([128, CHUNK_MAX, KC, BLK], mybir.dt.float16,
                                  tag="xh", name=f"xh{ci}")
                x8_sb = xbuf.tile([128, 2, CHUNK_MAX, KC, BLK],
                                  mybir.dt.float8e4, tag="x8", name=f"x8{ci}")
                nc.sync.dma_start(
                    out=xh_sb[:, 0:nb], in_=xh_v[:, b0:b0 + nb])
                nc.gpsimd.dma_start(
                    out=x8_sb[:, 1, 0:nb], in_=xl_v[:, b0:b0 + nb])
                # fill fp8 pair plane 0 with fp8(xh); split across DVE and ACT
                half = KC // 2
                nc.vector.tensor_copy(
                    out=x8_sb[:, 0, 0:nb, 0:half, :],
                    in_=xh_sb[:, 0:nb, 0:half, :])
                nc.scalar.activation(
                    out=x8_sb[:, 0, 0:nb, half:KC, :],
                    in_=xh_sb[:, 0:nb, half:KC, :],
                    func=mybir.ActivationFunctionType.Copy)

                # "phase": all fp16 matmuls of the chunk first (they only
                # need xh) so the PE covers the fp8-cast latency; "block":
                # interleave per block for steady PSUM-bank churn.
                lg = {
                    s: [
                        pslog.tile([128, BLK], mybir.dt.float32, tag="lg",
                                   name=f"lg{b0 + s}_{j}")
                        for j in range(n_etiles)
                    ]
                    for s in range(nb)
                }
                if MMORD == "phase":
                    for k in range(KC):
                        for s in range(nb):
                            for j in range(n_etiles):
                                nc.tensor.matmul(
                                    lg[s][j], wh2_t[:, j, k], xh_sb[:, s, k, :],
                                    start=(k == 0), stop=False,
                                )
                    for k in range(KC):
                        for s in range(nb):
                            for j in range(n_etiles):
                                nc.tensor.matmul(
                                    lg[s][j], w8_t[:, j, k], x8_sb[:, :, s, k, :],
                                    start=False, stop=(k == KC - 1),
                                    perf_mode=mybir.MatmulPerfMode.DoubleRow,
                                )
                else:
                    for s in range(nb):
                        for j in range(n_etiles):
                            for k in range(KC):
                                nc.tensor.matmul(
                                    lg[s][j], wh2_t[:, j, k], xh_sb[:, s, k, :],
                                    start=(k == 0), stop=False,
                                )
                            for k in range(KC):
                                nc.tensor.matmul(
                                    lg[s][j], w8_t[:, j, k], x8_sb[:, :, s, k, :],
                                    start=False, stop=(k == KC - 1),
                                    perf_mode=mybir.MatmulPerfMode.DoubleRow,
                                )
                for s in range(nb):
                    bi = b0 + s
                    s_ts = []
                    for j in range(n_etiles):
                        s_t = spool.tile([128, BLK], mybir.dt.bfloat16,
                                         tag="sg", name=f"sg{bi}_{j}")
                        # pred = 1[2^12 x@W > -2^12 b]
                        nc.vector.tensor_scalar(
                            s_t, lg[s][j], bv2_t[:, j:j + 1], None,
                            mybir.AluOpType.is_gt,
                        )
                        s_ts.append(s_t)
                    pending.append((s_ts, bi))
                    while len(pending) > PEND:
                        flush_stage2()
                b0 += nb
            while pending:
                flush_stage2()

    nc.compile()
    return nc


def _prep_inputs(x, W, b, alphas):
    """Host-side prep: estimator selection, transposes, hi/lo splits."""
    t_full = np.trunc(alphas.astype(np.float32)).astype(np.float32)
    nz = np.flatnonzero(t_full)
    n_etiles = max(1, math.ceil(len(nz) / 128))
    e_pad = n_etiles * 128

    W_sel = np.zeros((e_pad, F_DIM), np.float32)
    b_sel = np.zeros((e_pad,), np.float32)
    t_sel = np.zeros((e_pad,), np.float32)
    if len(nz):
        W_sel[: len(nz)] = W[nz]
        b_sel[: len(nz)] = b[nz]
        t_sel[: len(nz)] = t_full[nz]

    # [n_etiles, F, 128] stationary layout (partition = feature), then
    # repacked to the SBUF tile layout [128p, j, k, e]
    w_fe = W_sel.T.reshape(F_DIM, n_etiles, 128).transpose(1, 0, 2)
    wh = w_fe.astype(np.float16)
    # exact power-of-two scale: fp16(W) * 2^12 has no extra rounding
    wh2 = (wh.astype(np.float32) * XL_SCALE).astype(np.float16)
    wl32 = w_fe - wh.astype(np.float32)
    w8 = np.empty((n_etiles, F_DIM, 2, 128), F8E4)
    w8[..., 0, :] = (wl32 * XL_SCALE).astype(F8E4)
    w8[..., 1, :] = wh.astype(np.float32).astype(F8E4)
    # [j, (k p), ...] -> [p, j, k, ...]
    wh2_p = np.ascontiguousarray(
        wh2.reshape(n_etiles, KC, 128, 128).transpose(2, 0, 1, 3))
    w8_p = np.ascontiguousarray(
        w8.reshape(n_etiles, KC, 128, 2, 128).transpose(2, 0, 1, 3, 4))

    bv2 = np.ascontiguousarray(
        (-(b_sel * XL_SCALE)).reshape(n_etiles, 128).T
    ).astype(np.float32)
    tvb = np.ascontiguousarray(
        np.broadcast_to(
            t_sel.reshape(n_etiles, 128, 1), (n_etiles, 128, 32)
        ).transpose(1, 0, 2)
    ).astype(BF16)

    xT = np.ascontiguousarray(x.T.astype(np.float32))  # [F, N]
    xh = xT.astype(np.float16)
    xl = ((xT - xh.astype(np.float32)) * XL_SCALE).astype(F8E4)

    in_maps = []
    for c in range(N_CORES):
        sl = slice(c * NS, (c + 1) * NS)
        # repack: [F=(k p), n] -> [p, block, k, n] so every (p, block) line
        # is contiguous and any run of blocks is one large-descriptor DMA
        xh_c = xh[:, sl].reshape(KC, 128, NBLK, BLK).transpose(1, 2, 0, 3)
        xl_c = xl[:, sl].reshape(KC, 128, NBLK, BLK).transpose(1, 2, 0, 3)
        in_maps.append({
            "xh": np.ascontiguousarray(xh_c),
            "xl": np.ascontiguousarray(xl_c),
            "wh2": wh2_p, "w8": w8_p, "bv2": bv2, "tvb": tvb,
        })
    return n_etiles, in_maps


def kernel(x, W, b, alphas, _trace=False, _trace_kwargs=None):
    n_etiles, in_maps = _prep_inputs(
        np.asarray(x), np.asarray(W), np.asarray(b), np.asarray(alphas)
    )
    cache_key = (n_etiles, _SCHED, XBUFS, SPOOL, PSLOG, PEND, WARM, WARMC, MMORD)
    nc = _program_cache.get(cache_key)
    if nc is None:
        nc = _build(n_etiles)
        _program_cache[cache_key] = nc

    kwargs = {}
    if _trace:
        kwargs["trace"] = True
        kwargs.update(_trace_kwargs or {})
    res = run_bass_kernel_spmd(nc, in_maps, core_ids=list(range(N_CORES)), **kwargs)
    out = np.concatenate([res.results[c]["out"] for c in range(N_CORES)])
    if _trace:
        kernel.last_results = res
    return out.astype(np.float32)
